# revision 1
# baseline (speedup 1.0000x reference)
"""DISK keypoint detection on 8 Trainium2 NeuronCores (Bass/Tile).

kernel(unet_output[4,129,512,768] f32) -> (keypoints [4,2048,2] i32,
scores [4,2048] f32, descriptors [4,2048,128] f32), matching:
  5x5 maxpool NMS on the heatmap channel -> top-2048 over h*w per image
  (desc value, ties by index) -> descriptor gather -> L2 normalize.

Sharding: pure data parallel; core k = image k//2, row-half k%2.
Launch 1 does NMS + per-column-strip top-40 candidate extraction
(max8/max_index/match_replace); the host merges the two halves' shortlists
into the exact per-image top-2048 (with an astronomically-unlikely saturation
fallback). Launch 2 streams each descriptor half through SBUF in
[128ch, 16384px] tiles and gathers keypoint columns per tile with ap_gather
(out-of-tile slots hit a zero sentinel column, so per-tile gathers sum),
then L2-normalizes on-chip (PE sum-of-squares + exact-reciprocal Newton).
"""

import numpy as np
import concourse.bass as bass
import concourse.bacc as bacc
import concourse.mybir as mybir
import concourse.tile as tile
from concourse.bass_utils import run_bass_kernel_spmd

F32 = mybir.dt.float32
U16 = mybir.dt.uint16
I16 = mybir.dt.int16

NEG = -1.0e30
H, W = 512, 768
HALF = 256          # rows per core
D = 128             # descriptor channels
K = 2048            # keypoints per image
R = 5               # max8 extraction rounds -> top-40 per strip
NCAND = 8 * R
SC = 6              # own columns per strip (128 strips x 6 = 768)
SCP = SC + 4        # strip columns incl. 2+2 halo
SR = HALF + 4       # strip rows incl. 2+2 halo
NTILE = 12
TPIX = 16384        # 12*16384 = 196608 = 256*768
NSLOT = 1280        # padded keypoint slots per core in launch 2

_PROFILE = False    # test harness sets True to collect NTFF exec times
_EXEC_NS = {}


def build_k1():
    nc = bacc.Bacc("TRN2", target_bir_lowering=False, debug=False, num_devices=8)
    hm = nc.dram_tensor("hm", [128, SCP * SR], F32, kind="ExternalInput")
    cv = nc.dram_tensor("cv", [128, NCAND], F32, kind="ExternalOutput")
    ci = nc.dram_tensor("ci", [128, NCAND], U16, kind="ExternalOutput")

    def ap3(t, c0, ncols, r0, nrows, rowstride=SR):
        a = t[:]
        return bass.AP(a.tensor, a.offset + c0 * rowstride + r0,
                       [a.ap[0], [rowstride, ncols], [1, nrows]])

    with tile.TileContext(nc) as tc:
        with tc.tile_pool(name="p", bufs=1) as pool:
            x = pool.tile([128, SCP * SR], F32)        # [10, 260] per strip
            nc.sync.dma_start(x[:], hm.ap())

            a1 = pool.tile([128, SCP * (SR - 2)], F32)
            a2 = pool.tile([128, SCP * (SR - 2)], F32)
            nc.vector.tensor_max(ap3(a1, 0, SCP, 0, SR - 2, SR - 2),
                                 ap3(x, 0, SCP, 0, SR - 2),
                                 ap3(x, 0, SCP, 1, SR - 2))
            nc.vector.tensor_max(ap3(a2, 0, SCP, 0, SR - 2, SR - 2),
                                 ap3(a1, 0, SCP, 0, SR - 2, SR - 2),
                                 ap3(x, 0, SCP, 2, SR - 2))
            v5 = pool.tile([128, SCP * HALF], F32)
            nc.vector.tensor_max(ap3(v5, 0, SCP, 0, HALF, HALF),
                                 ap3(a2, 0, SCP, 0, HALF, SR - 2),
                                 ap3(a2, 0, SCP, 2, HALF, SR - 2))
            b1 = pool.tile([128, (SCP - 2) * HALF], F32)
            b3 = pool.tile([128, (SCP - 2) * HALF], F32)
            nc.vector.tensor_max(ap3(b1, 0, SCP - 2, 0, HALF, HALF),
                                 ap3(v5, 0, SCP - 2, 0, HALF, HALF),
                                 ap3(v5, 1, SCP - 2, 0, HALF, HALF))
            nc.vector.tensor_max(ap3(b3, 0, SCP - 2, 0, HALF, HALF),
                                 ap3(b1, 0, SCP - 2, 0, HALF, HALF),
                                 ap3(v5, 2, SCP - 2, 0, HALF, HALF))
            h5 = pool.tile([128, SC * HALF], F32)
            nc.vector.tensor_max(ap3(h5, 0, SC, 0, HALF, HALF),
                                 ap3(b3, 0, SC, 0, HALF, HALF),
                                 ap3(b3, 2, SC, 0, HALF, HALF))

            xo = ap3(x, 2, SC, 2, HALF)
            eq = pool.tile([128, SC * HALF], F32)
            nc.vector.tensor_tensor(ap3(eq, 0, SC, 0, HALF, HALF), xo,
                                    ap3(h5, 0, SC, 0, HALF, HALF),
                                    op=mybir.AluOpType.is_equal)
            sup_a = pool.tile([128, SC * HALF], F32)
            nc.vector.tensor_mul(ap3(sup_a, 0, SC, 0, HALF, HALF),
                                 ap3(eq, 0, SC, 0, HALF, HALF), xo)
            sup_b = pool.tile([128, SC * HALF], F32)

            vt = pool.tile([128, NCAND], F32)
            it = pool.tile([128, NCAND], U16)
            cur, nxt = sup_a, sup_b
            for r in range(R):
                vs = vt[:, 8 * r:8 * r + 8]
                nc.vector.max(vs, cur[:])
                nc.vector.max_index(it[:, 8 * r:8 * r + 8], vs, cur[:])
                if r < R - 1:
                    nc.vector.match_replace(nxt[:], vs, cur[:], NEG)
                    cur, nxt = nxt, cur
            nc.sync.dma_start(cv.ap(), vt[:])
            nc.sync.dma_start(ci.ap(), it[:])
    nc.compile()
    return nc


def build_k2():
    nc = bacc.Bacc("TRN2", target_bir_lowering=False, debug=False, num_devices=8)
    desc = nc.dram_tensor("desc", [D, HALF * W], F32, kind="ExternalInput")
    idx = nc.dram_tensor("idx", [128, NTILE * 80], I16, kind="ExternalInput")
    dn = nc.dram_tensor("dn", [D, NSLOT], F32, kind="ExternalOutput")

    with tile.TileContext(nc) as tc:
        with (
            tc.tile_pool(name="stream", bufs=2) as spool,
            tc.tile_pool(name="gp", bufs=2) as gpool,
            tc.tile_pool(name="small", bufs=1) as pool,
            tc.tile_pool(name="psum", bufs=2, space="PSUM") as psum,
        ):
            idx_sb = pool.tile([128, NTILE * 80], I16, tag="idx")
            nc.sync.dma_start(idx_sb[:], idx.ap())

            acc_a = pool.tile([D, NSLOT], F32, tag="acca")
            acc_b = pool.tile([D, NSLOT], F32, tag="accb")
            cur, nxt = acc_a, acc_b
            for t in range(NTILE):
                buf = spool.tile([128, TPIX + 1], F32)
                nc.vector.memset(buf[:, TPIX:TPIX + 1], 0.0)
                nc.sync.dma_start(buf[:, 0:TPIX],
                                  desc.ap()[:, t * TPIX:(t + 1) * TPIX])
                g = gpool.tile([D, NSLOT], F32, tag="g")
                nc.gpsimd.ap_gather(
                    g[:], buf[:], idx_sb[:, t * 80:(t + 1) * 80],
                    channels=128, num_elems=TPIX + 1, d=1, num_idxs=NSLOT)
                if t == 0:
                    nc.vector.tensor_copy(cur[:], g[:])
                else:
                    nc.vector.tensor_add(nxt[:], cur[:], g[:])
                    cur, nxt = nxt, cur
            acc = cur

            sq = pool.tile([D, NSLOT], F32, tag="sq")
            nc.vector.tensor_mul(sq[:], acc[:], acc[:])
            ones = pool.tile([D, 1], F32, tag="ones")
            nc.vector.memset(ones[:], 1.0)
            ssq_ps = psum.tile([1, NSLOT], F32, tag="ssq")
            for j in range(0, NSLOT, 512):
                n = min(512, NSLOT - j)
                nc.tensor.matmul(ssq_ps[:, j:j + n], ones[:], sq[:, j:j + n],
                                 start=True, stop=True)
            ssq = pool.tile([1, NSLOT], F32, tag="ssqs")
            nc.vector.tensor_scalar_max(ssq[:], ssq_ps[:], 1e-24)
            nrm = pool.tile([1, NSLOT], F32, tag="nrm")
            nc.scalar.activation(nrm[:], ssq[:], mybir.ActivationFunctionType.Sqrt)
            t1 = pool.tile([1, NSLOT], F32, tag="t1")
            nc.vector.reciprocal(t1[:], nrm[:])
            nc.vector.tensor_mul(t1[:], t1[:], ssq[:])
            nc.vector.tensor_add(t1[:], t1[:], nrm[:])
            nc.vector.tensor_scalar_mul(t1[:], t1[:], 0.5)
            nc.vector.tensor_scalar_max(t1[:], t1[:], 1e-12)
            r0 = pool.tile([1, NSLOT], F32, tag="r0")
            nc.vector.reciprocal(r0[:], t1[:])
            ones1 = pool.tile([1, 128], F32, tag="ones1")
            nc.vector.memset(ones1[:], 1.0)
            rb_ps = psum.tile([D, 512], F32, tag="rb")
            out_sb = pool.tile([D, NSLOT], F32, tag="out")
            for j in range(0, NSLOT, 512):
                n = min(512, NSLOT - j)
                nc.tensor.matmul(rb_ps[:, :n], ones1[:], r0[:, j:j + n],
                                 start=True, stop=True)
                nc.vector.tensor_mul(out_sb[:, j:j + n], acc[:, j:j + n],
                                     rb_ps[:, :n])
            nc.sync.dma_start(dn.ap(), out_sb[:])
    nc.compile()
    return nc


_NC1 = None
_NC2 = None


def _get_ncs():
    global _NC1, _NC2
    if _NC1 is None:
        _NC1 = build_k1()
        _NC2 = build_k2()
    return _NC1, _NC2


def np_reference_single(img):
    """Full numpy reference for one image [129, 512, 768] (fallback path)."""
    desc = img[:D]
    heat = img[D]
    pad = np.full((H + 4, W + 4), -np.inf, np.float32)
    pad[2:-2, 2:-2] = heat
    pooled = np.full((H, W), -np.inf, np.float32)
    for dy in range(5):
        for dx in range(5):
            np.maximum(pooled, pad[dy:dy + H, dx:dx + W], out=pooled)
    sup = np.where(heat == pooled, heat, 0.0).astype(np.float32)
    flat = sup.reshape(-1)
    order = np.lexsort((np.arange(flat.size), -flat))[:K]
    scores = flat[order]
    y = (order // W).astype(np.int32)
    x = (order % W).astype(np.int32)
    d = desc[:, y, x].T.astype(np.float32)
    n = np.maximum(np.sqrt((d ** 2).sum(-1, keepdims=True)), 1e-12)
    return (np.stack([x, y], -1).astype(np.int32), scores.astype(np.float32),
            (d / n).astype(np.float32))


def prep_k1_inputs(unet):
    ins = []
    for k in range(8):
        b, hf = k // 2, k % 2
        r0 = HALF * hf
        hp = np.full((W + 4, SR), NEG, np.float32)
        rlo, rhi = max(0, r0 - 2), min(H, r0 + HALF + 2)
        hp[2:W + 2, rlo - (r0 - 2):rhi - (r0 - 2)] = unet[b, D, rlo:rhi, :].T
        strips = np.lib.stride_tricks.as_strided(
            hp, (128, SCP, SR), (SC * SR * 4, SR * 4, 4))
        ins.append({"hm": np.ascontiguousarray(strips).reshape(128, SCP * SR)})
    return ins


def merge_candidates(unet, k1_results):
    kps = np.zeros((4, K, 2), np.int32)
    scs = np.zeros((4, K), np.float32)
    yx = []
    for b in range(4):
        gi_all, v_all, strip_all = [], [], []
        for hf in range(2):
            res = k1_results[2 * b + hf]
            cv = res["cv"]                      # [128, 40]
            ci = res["ci"].astype(np.int64)     # strip-local: col*256 + row
            p = np.arange(128)[:, None]
            col = SC * p + ci // HALF
            y = HALF * hf + ci % HALF
            gi_all.append((y * W + col).reshape(-1))
            v_all.append(cv.reshape(-1))
            strip_all.append(np.broadcast_to(p + 128 * hf, cv.shape).reshape(-1))
        gi = np.concatenate(gi_all)
        v = np.concatenate(v_all)
        strip = np.concatenate(strip_all)
        pos = v > 0.0
        ok = bool(pos.sum() >= K)
        if ok:
            gi_p, v_p = gi[pos], v[pos]
            order = np.lexsort((gi_p, -v_p))[:K]
            t_k = v_p[order[-1]]
            sat = np.bincount(strip[(v >= t_k)], minlength=256)
            ok = bool(sat.max(initial=0) < NCAND)
        if not ok:
            kp, sc, dref = np_reference_single(unet[b])
            kps[b], scs[b] = kp, sc
            yx.append((kp[:, 1].astype(np.int64), kp[:, 0].astype(np.int64),
                       False, dref))
            continue
        g = gi_p[order]
        y, x = g // W, g % W
        kps[b, :, 0], kps[b, :, 1] = x, y
        scs[b] = v_p[order]
        yx.append((y, x, True, None))
    return kps, scs, yx


def prep_k2_inputs(unet, yx):
    ins, slot_maps = [], []
    for k in range(8):
        b, hf = k // 2, k % 2
        y, x, ok, _ = yx[b]
        r0 = HALF * hf
        dsc = np.ascontiguousarray(
            unet[b, :D, r0:r0 + HALF, :].reshape(D, HALF * W))
        if ok:
            slots = np.where((y >= r0) & (y < r0 + HALF))[0]
            if len(slots) > NSLOT:      # ~11-sigma event; host fallback
                kp, sc, dref = np_reference_single(unet[b])
                yx[b] = (kp[:, 1].astype(np.int64), kp[:, 0].astype(np.int64),
                         False, dref)
                slots = np.array([], np.int64)
        else:
            slots = np.array([], np.int64)
        lidx = (y[slots] - r0) * W + x[slots]
        arr = np.full((NTILE, NSLOT), TPIX, np.int64)   # sentinel = zero col
        lt = lidx[None, :] - TPIX * np.arange(NTILE)[:, None]
        intile = (lt >= 0) & (lt < TPIX)
        arr[:, :len(slots)] = np.where(intile, lt, TPIX)
        wrapped = arr.reshape(NTILE, 80, 16).transpose(0, 2, 1)   # [12,16,80]
        idx16 = np.zeros((128, NTILE * 80), np.int16)
        for grp in range(8):
            idx16[16 * grp:16 * grp + 16] = (
                wrapped.transpose(1, 0, 2).reshape(16, NTILE * 80))
        ins.append({"desc": dsc, "idx": idx16})
        slot_maps.append(slots)
    return ins, slot_maps


def kernel(unet_output):
    unet = np.asarray(unet_output, dtype=np.float32)
    assert unet.shape == (4, D + 1, H, W), unet.shape
    nc1, nc2 = _get_ncs()
    cores = list(range(8))

    k1_ins = prep_k1_inputs(unet)
    r1 = run_bass_kernel_spmd(nc1, k1_ins, cores, trace=_PROFILE)
    if _PROFILE:
        _EXEC_NS["k1"] = r1.exec_time_ns
    kps, scs, yx = merge_candidates(unet, r1.results)

    k2_ins, slot_maps = prep_k2_inputs(unet, yx)
    r2 = run_bass_kernel_spmd(nc2, k2_ins, cores, trace=_PROFILE)
    if _PROFILE:
        _EXEC_NS["k2"] = r2.exec_time_ns

    d_out = np.zeros((4, K, D), np.float32)
    for b in range(4):
        y, x, ok, dref = yx[b]
        if not ok:
            d_out[b] = dref
            continue
        for hf in range(2):
            k = 2 * b + hf
            slots = slot_maps[k]
            dn = r2.results[k]["dn"]
            d_out[b, slots, :] = dn[:, :len(slots)].T
    return kps, scs, d_out


# revision 11
# speedup vs baseline: 1.6382x; 1.6382x over previous
"""DISK keypoint detection on 8 Trainium2 NeuronCores (Bass/Tile).

kernel(unet_output[4,129,512,768] f32) -> (keypoints [4,2048,2] i32,
scores [4,2048] f32, descriptors [4,2048,128] f32), matching:
  5x5 maxpool NMS on the heatmap channel -> top-2048 over h*w per image
  (desc value, ties by index) -> descriptor gather -> L2 normalize.

Sharding: pure data parallel; core k = image k//2, row-half k%2.

Launch 1: NMS via separable shifted-max on a host-prepared column-strip
layout (partition = 6 image columns + halos; both pool directions are
free-dim shifts), then per-quarter-strip top-16 candidate extraction with
max8/max_index/match_replace. The host merges the per-half shortlists into
the exact per-image top-2048; astronomically-unlikely extraction saturation
is detected exactly and falls back to a full numpy recompute of that image.

Launch 2: streams each descriptor half through SBUF in [128ch, 16384px]
tiles and gathers keypoint columns per tile with ap_gather. Keypoint slots
are pixel-sorted and spread host-side so each tile only needs a static
320-slot window; out-of-tile slots index a zero sentinel column, so window
contributions just add. L2 normalization runs on-chip (PE sum-of-squares,
LUT sqrt + one exact-reciprocal Newton step, PE broadcast).
"""

import numpy as np
import concourse.bass as bass
import concourse.bacc as bacc
import concourse.mybir as mybir
import concourse.tile as tile
from concourse import library_config
from concourse.bass_utils import run_bass_kernel_spmd

F32 = mybir.dt.float32
U16 = mybir.dt.uint16
I16 = mybir.dt.int16

NEG = -1.0e30
H, W = 512, 768
HALF = 256          # rows per core
D = 128             # descriptor channels
K = 2048            # keypoints per image
NQ = 4              # quarter-strips per strip
QLEN = 1536 // NQ   # elements per quarter (384)
RQ = 2              # max8 rounds per quarter -> top-16
QCAND = 8 * RQ      # candidates per quarter
SC = 6              # own columns per strip (128 strips x 6 = 768)
SCP = SC + 4        # strip columns incl. 2+2 halo
SR = HALF + 4       # strip rows incl. 2+2 halo
NTILE = 16
TPIX = 12288        # 16*12288 = 196608 = 256*768
NSLOT = 1280        # padded keypoint slots per core in launch 2
WSZ = 288           # gather window per tile (pixel-sorted slot positions)
WSTEP = NSLOT // NTILE
WINS = [min(max(WSTEP * t - 104, 0), NSLOT - WSZ) for t in range(NTILE)]
# after add(t), slot positions [WINS[t], WINS[t+1]) are final (pixel-sorted
# coverage: position p < WINS[s] implies p's pixel is in a tile before s)
FINS = [(WINS[t], WINS[t + 1] if t + 1 < NTILE else NSLOT)
        for t in range(NTILE)]

_PROFILE = False    # test harness sets True to collect NTFF exec times
_EXEC_NS = {}


def build_k1():
    nc = bacc.Bacc("TRN2", target_bir_lowering=False, debug=False, num_devices=8)
    hm = nc.dram_tensor("hm", [128, SCP * SR], F32, kind="ExternalInput")
    cv = nc.dram_tensor("cv", [128, NQ * QCAND], F32, kind="ExternalOutput")
    ci = nc.dram_tensor("ci", [128, NQ * QCAND], U16, kind="ExternalOutput")

    def ap3(t, c0, ncols, r0, nrows, rowstride=SR, p0=0, np_=128):
        a = t[:]
        return bass.AP(
            a.tensor, a.offset + p0 * a.ap[0][0] + c0 * rowstride + r0,
            [[a.ap[0][0], np_], [rowstride, ncols], [1, nrows]])

    def split_max(out_t, a_spec, b_spec):
        (ta, ca, ra, rsa), (tb, cb, rb, rsb) = a_spec, b_spec
        oc, orw, ors = out_t[1], out_t[2], out_t[3]
        nc.vector.tensor_max(
            ap3(out_t[0], oc, out_t[4], orw, out_t[5], ors),
            ap3(ta, ca, out_t[4], ra, out_t[5], rsa),
            ap3(tb, cb, out_t[4], rb, out_t[5], rsb))

    with tile.TileContext(nc) as tc:
        with tc.tile_pool(name="p", bufs=1) as pool:
            x = pool.tile([128, SCP * SR], F32)        # [10, 260] per strip
            nc.sync.dma_start(x[:], hm.ap())

            a1 = pool.tile([128, SCP * (SR - 2)], F32)
            a2 = pool.tile([128, SCP * (SR - 2)], F32)
            # vertical 3-max then 5-max (rows are the contiguous free axis)
            split_max((a1, 0, 0, SR - 2, SCP, SR - 2),
                      (x, 0, 0, SR), (x, 0, 1, SR))
            split_max((a2, 0, 0, SR - 2, SCP, SR - 2),
                      (a1, 0, 0, SR - 2), (x, 0, 2, SR))
            v5 = pool.tile([128, SCP * HALF], F32)
            split_max((v5, 0, 0, HALF, SCP, HALF),
                      (a2, 0, 0, SR - 2), (a2, 0, 2, SR - 2))
            # horizontal 3-max then 5-max (columns stride by the row length)
            b1 = pool.tile([128, (SCP - 2) * HALF], F32)
            b3 = pool.tile([128, (SCP - 2) * HALF], F32)
            split_max((b1, 0, 0, HALF, SCP - 2, HALF),
                      (v5, 0, 0, HALF), (v5, 1, 0, HALF))
            split_max((b3, 0, 0, HALF, SCP - 2, HALF),
                      (b1, 0, 0, HALF), (v5, 2, 0, HALF))
            h5 = pool.tile([128, SC * HALF], F32)
            split_max((h5, 0, 0, HALF, SC, HALF),
                      (b3, 0, 0, HALF), (b3, 2, 0, HALF))

            # suppress own region: sup = (x_own == h5) ? x_own : 0
            # (is_equal is not a valid Pool opcode -> DVE only)
            eq = pool.tile([128, SC * HALF], F32)
            sup_a = pool.tile([128, SC * HALF], F32)
            xo = ap3(x, 2, SC, 2, HALF)
            nc.vector.tensor_tensor(ap3(eq, 0, SC, 0, HALF, HALF), xo,
                                    ap3(h5, 0, SC, 0, HALF, HALF),
                                    op=mybir.AluOpType.is_equal)
            nc.vector.tensor_mul(ap3(sup_a, 0, SC, 0, HALF, HALF),
                                 ap3(eq, 0, SC, 0, HALF, HALF), xo)
            sup_b = pool.tile([128, SC * HALF], F32)

            vt = pool.tile([128, NQ * QCAND], F32)
            it = pool.tile([128, NQ * QCAND], U16)
            for q in range(NQ):
                cur, nxt = sup_a, sup_b
                for r in range(RQ):
                    vs = vt[:, QCAND * q + 8 * r:QCAND * q + 8 * r + 8]
                    cs = cur[:, QLEN * q:QLEN * (q + 1)]
                    nc.vector.max(vs, cs)
                    nc.vector.max_index(
                        it[:, QCAND * q + 8 * r:QCAND * q + 8 * r + 8], vs, cs)
                    if r < RQ - 1:
                        nc.vector.match_replace(
                            nxt[:, QLEN * q:QLEN * (q + 1)], vs, cs, NEG)
                        cur, nxt = nxt, cur
            nc.sync.dma_start(cv.ap(), vt[:])
            nc.sync.dma_start(ci.ap(), it[:])
    nc.compile()
    return nc


def build_k2():
    nc = bacc.Bacc("TRN2", target_bir_lowering=False, debug=False, num_devices=8)
    desc = nc.dram_tensor("desc", [D, HALF * W], F32, kind="ExternalInput")
    idx = nc.dram_tensor("idx", [128, NTILE * WSZ // 16], I16,
                         kind="ExternalInput")
    dn = nc.dram_tensor("dn", [D, NSLOT], F32, kind="ExternalOutput")
    IW = WSZ // 16      # wrapped idx columns per tile

    with tile.TileContext(nc) as tc:
        with (
            tc.tile_pool(name="stream", bufs=1) as spool,
            tc.tile_pool(name="gp", bufs=2) as gpool,
            tc.tile_pool(name="fin", bufs=2) as fpool,
            tc.tile_pool(name="small", bufs=1) as pool,
            tc.tile_pool(name="psum", bufs=2, space="PSUM") as psum,
        ):
            # pull the Q7 ap_gather library in parallel with the first DMAs
            nc.gpsimd.load_library(library_config.ap_gather)
            idx_sb = pool.tile([128, NTILE * IW], I16, tag="idx")
            nc.gpsimd.dma_start(idx_sb[:], idx.ap())

            bufs = [spool.tile([128, TPIX + 1], F32, tag=f"buf{i}",
                               name=f"buf{i}")
                    for i in range(3)]
            for bf in bufs:
                nc.vector.memset(bf[:, TPIX:TPIX + 1], 0.0)
            ones = pool.tile([D, 1], F32, tag="ones")
            nc.vector.memset(ones[:], 1.0)
            ones1 = pool.tile([1, 128], F32, tag="ones1")
            nc.vector.memset(ones1[:], 1.0)
            acc = pool.tile([D, NSLOT], F32, tag="acc")
            nc.vector.memset(acc[:], 0.0)

            def finalize(f0, f1):
                """L2-normalize final slot positions [f0, f1) and write out."""
                n = f1 - f0
                sqf = fpool.tile([D, n], F32, tag="sqf", name="sqf")
                nc.vector.tensor_mul(sqf[:], acc[:, f0:f1], acc[:, f0:f1])
                ssq_ps = psum.tile([1, n], F32, tag="ssq", name="ssqps")
                # split matmuls at PSUM bank boundaries of the psum tile
                for j in range(0, n, 512):
                    m = min(512, n - j)
                    nc.tensor.matmul(ssq_ps[:, j:j + m], ones[:],
                                     sqf[:, j:j + m], start=True, stop=True)
                ssq = fpool.tile([1, n], F32, tag="ssqs", name="ssqs")
                nc.vector.tensor_scalar_max(ssq[:], ssq_ps[:], 1e-24)
                nrm = fpool.tile([1, n], F32, tag="nrm", name="nrm")
                nc.scalar.activation(nrm[:], ssq[:],
                                     mybir.ActivationFunctionType.Sqrt)
                # one Newton step in rsqrt form (one exact recip):
                # r = recip(sqrt_lut(s)); r *= 1.5 - 0.5*s*r^2
                r0 = fpool.tile([1, n], F32, tag="r0", name="r0")
                nc.vector.reciprocal(r0[:], nrm[:])
                t1 = fpool.tile([1, n], F32, tag="t1", name="t1")
                nc.vector.tensor_mul(t1[:], r0[:], r0[:])
                nc.vector.tensor_mul(t1[:], t1[:], ssq[:])
                nc.vector.tensor_scalar(t1[:], t1[:], -0.5, 1.5,
                                        op0=mybir.AluOpType.mult,
                                        op1=mybir.AluOpType.add)
                nc.vector.tensor_mul(r0[:], r0[:], t1[:])
                rb_ps = psum.tile([D, n], F32, tag="rb", name="rbps")
                outf = fpool.tile([D, n], F32, tag="outf", name="outf")
                for j in range(0, n, 512):
                    m = min(512, n - j)
                    nc.tensor.matmul(rb_ps[:, j:j + m], ones1[:],
                                     r0[:, j:j + m], start=True, stop=True)
                    nc.vector.tensor_mul(outf[:, j:j + m],
                                         acc[:, f0 + j:f0 + j + m],
                                         rb_ps[:, j:j + m])
                nc.sync.dma_start(dn.ap()[:, f0:f1], outf[:])

            for t in range(NTILE):
                buf = bufs[t % 3]
                nc.sync.dma_start(buf[:, 0:TPIX],
                                  desc.ap()[:, t * TPIX:(t + 1) * TPIX])
                g = gpool.tile([D, WSZ], F32, tag="g")
                nc.gpsimd.ap_gather(
                    g[:], buf[:], idx_sb[:, t * IW:(t + 1) * IW],
                    channels=128, num_elems=TPIX + 1, d=1, num_idxs=WSZ)
                o = WINS[t]
                nc.vector.tensor_add(acc[:, o:o + WSZ], acc[:, o:o + WSZ], g[:])
                f0, f1 = FINS[t]
                if f1 > f0:
                    finalize(f0, f1)
    nc.compile()
    return nc


_NC1 = None
_NC2 = None


def _get_ncs():
    global _NC1, _NC2
    if _NC1 is None:
        _NC1 = build_k1()
        _NC2 = build_k2()
    return _NC1, _NC2


def np_reference_single(img):
    """Full numpy reference for one image [129, 512, 768] (fallback path)."""
    desc = img[:D]
    heat = img[D]
    pad = np.full((H + 4, W + 4), -np.inf, np.float32)
    pad[2:-2, 2:-2] = heat
    pooled = np.full((H, W), -np.inf, np.float32)
    for dy in range(5):
        for dx in range(5):
            np.maximum(pooled, pad[dy:dy + H, dx:dx + W], out=pooled)
    sup = np.where(heat == pooled, heat, 0.0).astype(np.float32)
    flat = sup.reshape(-1)
    order = np.lexsort((np.arange(flat.size), -flat))[:K]
    scores = flat[order]
    y = (order // W).astype(np.int32)
    x = (order % W).astype(np.int32)
    d = desc[:, y, x].T.astype(np.float32)
    n = np.maximum(np.sqrt((d ** 2).sum(-1, keepdims=True)), 1e-12)
    return (np.stack([x, y], -1).astype(np.int32), scores.astype(np.float32),
            (d / n).astype(np.float32))


def prep_k1_inputs(unet):
    ins = []
    for k in range(8):
        b, hf = k // 2, k % 2
        r0 = HALF * hf
        hp = np.full((W + 4, SR), NEG, np.float32)
        rlo, rhi = max(0, r0 - 2), min(H, r0 + HALF + 2)
        hp[2:W + 2, rlo - (r0 - 2):rhi - (r0 - 2)] = unet[b, D, rlo:rhi, :].T
        strips = np.lib.stride_tricks.as_strided(
            hp, (128, SCP, SR), (SC * SR * 4, SR * 4, 4))
        ins.append({"hm": np.ascontiguousarray(strips).reshape(128, SCP * SR)})
    return ins


def merge_candidates(unet, k1_results):
    kps = np.zeros((4, K, 2), np.int32)
    scs = np.zeros((4, K), np.float32)
    yx = []
    for b in range(4):
        gi_all, v_all, quart_all = [], [], []
        for hf in range(2):
            res = k1_results[2 * b + hf]
            cv = res["cv"]                      # [128, 64]
            ci = res["ci"].astype(np.int64)     # idx within quarter [0, 384)
            p = np.arange(128)[:, None]
            q = np.arange(NQ * QCAND)[None, :] // QCAND
            loc = QLEN * q + ci                 # strip-local: col*256 + row
            col = SC * p + loc // HALF
            y = HALF * hf + loc % HALF
            gi_all.append((y * W + col).reshape(-1))
            v_all.append(cv.reshape(-1))
            quart_all.append(
                np.broadcast_to((p + 128 * hf) * NQ + q, cv.shape).reshape(-1))
        gi = np.concatenate(gi_all)
        v = np.concatenate(v_all)
        quart = np.concatenate(quart_all)
        pos = v > 0.0
        ok = bool(pos.sum() >= K)
        if ok:
            gi_p, v_p = gi[pos], v[pos]
            order = np.lexsort((gi_p, -v_p))[:K]
            t_k = v_p[order[-1]]
            # a quarter whose 16 extracted candidates all clear t_k might
            # have held a 17th that also would -> cannot trust
            sat = np.bincount(quart[(v >= t_k)], minlength=256 * NQ)
            ok = bool(sat.max(initial=0) < QCAND)
        if not ok:
            kp, sc, dref = np_reference_single(unet[b])
            kps[b], scs[b] = kp, sc
            yx.append((kp[:, 1].astype(np.int64), kp[:, 0].astype(np.int64),
                       False, dref))
            continue
        g = gi_p[order]
        y, x = g // W, g % W
        kps[b, :, 0], kps[b, :, 1] = x, y
        scs[b] = v_p[order]
        yx.append((y, x, True, None))
    return kps, scs, yx


def prep_k2_inputs(unet, yx):
    """Pixel-sorted, spread keypoint slots + per-tile windowed gather idx."""
    IW = WSZ // 16
    wins = np.asarray(WINS)
    ins, slot_maps = [], []
    for k in range(8):
        b, hf = k // 2, k % 2
        y, x, ok, _ = yx[b]
        r0 = HALF * hf
        dsc = np.ascontiguousarray(
            unet[b, :D, r0:r0 + HALF, :].reshape(D, HALF * W))
        lidx_arr = np.full(NSLOT, -1, np.int64)
        rank_arr = np.full(NSLOT, -1, np.int64)
        if ok:
            ranks = np.where((y >= r0) & (y < r0 + HALF))[0]
            lidx = (y[ranks] - r0) * W + x[ranks]
            order = np.argsort(lidx, kind="stable")
            lidx, ranks = lidx[order], ranks[order]
            n = len(ranks)
            cov = n <= NSLOT
            if cov:
                pos = (np.arange(n) * NSLOT) // max(n, 1)
                lidx_arr[pos] = lidx
                rank_arr[pos] = ranks
                tjv = lidx // TPIX
                cov = bool(np.all((wins[tjv] <= pos) &
                                  (pos < wins[tjv] + WSZ)))
            if not cov:        # window overflow (~1e-5/run) -> exact fallback
                kp, sc, dref = np_reference_single(unet[b])
                yx[b] = (kp[:, 1].astype(np.int64), kp[:, 0].astype(np.int64),
                         False, dref)
                lidx_arr[:] = -1
                rank_arr[:] = -1
        arr = np.full((NTILE, WSZ), TPIX, np.int64)     # sentinel = zero col
        for t in range(NTILE):
            wj = lidx_arr[wins[t]:wins[t] + WSZ]
            lt = wj - TPIX * t
            arr[t] = np.where((wj >= 0) & (lt >= 0) & (lt < TPIX), lt, TPIX)
        wrapped = arr.reshape(NTILE, IW, 16).transpose(0, 2, 1)  # [12,16,IW]
        idx16 = np.zeros((128, NTILE * IW), np.int16)
        for grp in range(8):
            idx16[16 * grp:16 * grp + 16] = (
                wrapped.transpose(1, 0, 2).reshape(16, NTILE * IW))
        ins.append({"desc": dsc, "idx": idx16})
        slot_maps.append(rank_arr)
    return ins, slot_maps


def kernel(unet_output):
    unet = np.asarray(unet_output, dtype=np.float32)
    assert unet.shape == (4, D + 1, H, W), unet.shape
    nc1, nc2 = _get_ncs()
    cores = list(range(8))

    k1_ins = prep_k1_inputs(unet)
    r1 = run_bass_kernel_spmd(nc1, k1_ins, cores, trace=_PROFILE)
    if _PROFILE:
        _EXEC_NS["k1"] = r1.exec_time_ns
    kps, scs, yx = merge_candidates(unet, r1.results)

    k2_ins, slot_maps = prep_k2_inputs(unet, yx)
    r2 = run_bass_kernel_spmd(nc2, k2_ins, cores, trace=_PROFILE)
    if _PROFILE:
        _EXEC_NS["k2"] = r2.exec_time_ns

    d_out = np.zeros((4, K, D), np.float32)
    for b in range(4):
        y, x, ok, dref = yx[b]
        if not ok:
            d_out[b] = dref
            continue
        for hf in range(2):
            k = 2 * b + hf
            rank_arr = slot_maps[k]
            v = rank_arr >= 0
            dnb = r2.results[k]["dn"]
            d_out[b, rank_arr[v], :] = dnb[:, v].T
    return kps, scs, d_out


# revision 18
# speedup vs baseline: 1.7405x; 1.0624x over previous
"""DISK keypoint detection on 8 Trainium2 NeuronCores (Bass/Tile).

kernel(unet_output[4,129,512,768] f32) -> (keypoints [4,2048,2] i32,
scores [4,2048] f32, descriptors [4,2048,128] f32), matching:
  5x5 maxpool NMS on the heatmap channel -> top-2048 over h*w per image
  (desc value, ties by index) -> descriptor gather -> L2 normalize.

Sharding: pure data parallel; core k = image k//2, row-half k%2.

Launch 1: NMS via separable shifted-max on a host-prepared column-strip
layout (partition = 6 image columns + halos; both pool directions are
free-dim shifts), then per-quarter-strip top-16 candidate extraction with
max8/max_index/match_replace. The host merges the per-half shortlists into
the exact per-image top-2048; astronomically-unlikely extraction saturation
is detected exactly and falls back to a full numpy recompute of that image.

Launch 2: streams each descriptor half through SBUF in [128ch, 16384px]
tiles and gathers keypoint columns per tile with ap_gather. Keypoint slots
are pixel-sorted and spread host-side so each tile only needs a static
320-slot window; out-of-tile slots index a zero sentinel column, so window
contributions just add. L2 normalization runs on-chip (PE sum-of-squares,
LUT sqrt + one exact-reciprocal Newton step, PE broadcast).
"""

import numpy as np
import concourse.bass as bass
import concourse.bacc as bacc
import concourse.mybir as mybir
import concourse.tile as tile
from concourse import library_config
from concourse.bass_utils import run_bass_kernel_spmd

F32 = mybir.dt.float32
U16 = mybir.dt.uint16
I16 = mybir.dt.int16

NEG = -1.0e30
H, W = 512, 768
HALF = 256          # rows per core
D = 128             # descriptor channels
K = 2048            # keypoints per image
NQ = 4              # quarter-strips per strip
QLEN = 1536 // NQ   # elements per quarter (384)
RQ = 2              # max8 rounds per quarter -> top-16
QCAND = 8 * RQ      # candidates per quarter
SC = 6              # own columns per strip (128 strips x 6 = 768)
SCP = SC + 4        # strip columns incl. 2+2 halo
SR = HALF + 4       # strip rows incl. 2+2 halo
NTILE = 16
TPIX = 12288        # 16*12288 = 196608 = 256*768
NSLOT = 1280        # padded keypoint slots per core in launch 2
WSTEP = NSLOT // NTILE
# gather window (pixel-sorted slot positions) per tile; the last window is
# smaller so the after-the-stream finalize chain covers fewer slots
WSZS = [288] * (NTILE - 1) + [192]
WINS = [min(max(WSTEP * t - 104, 0), NSLOT - WSZS[t]) for t in range(NTILE)]
IWS = [w // 16 for w in WSZS]
IOFF = [sum(IWS[:t]) for t in range(NTILE + 1)]
# after add(t), slot positions [WINS[t], WINS[t+1]) are final (pixel-sorted
# coverage: position p < WINS[s] implies p's pixel is in a tile before s)
FINS = [(WINS[t], WINS[t + 1] if t + 1 < NTILE else NSLOT)
        for t in range(NTILE)]

_PROFILE = False    # test harness sets True to collect NTFF exec times
_EXEC_NS = {}


def build_k1():
    nc = bacc.Bacc("TRN2", target_bir_lowering=False, debug=False, num_devices=8)
    hm = nc.dram_tensor("hm", [128, SCP * SR], F32, kind="ExternalInput")
    cv = nc.dram_tensor("cv", [128, NQ * QCAND], F32, kind="ExternalOutput")
    ci = nc.dram_tensor("ci", [128, NQ * QCAND], U16, kind="ExternalOutput")

    def ap3(t, c0, ncols, r0, nrows, rowstride=SR, p0=0, np_=128):
        a = t[:]
        return bass.AP(
            a.tensor, a.offset + p0 * a.ap[0][0] + c0 * rowstride + r0,
            [[a.ap[0][0], np_], [rowstride, ncols], [1, nrows]])

    def split_max(out_t, a_spec, b_spec):
        (ta, ca, ra, rsa), (tb, cb, rb, rsb) = a_spec, b_spec
        oc, orw, ors = out_t[1], out_t[2], out_t[3]
        nc.vector.tensor_max(
            ap3(out_t[0], oc, out_t[4], orw, out_t[5], ors),
            ap3(ta, ca, out_t[4], ra, out_t[5], rsa),
            ap3(tb, cb, out_t[4], rb, out_t[5], rsb))

    with tile.TileContext(nc) as tc:
        with tc.tile_pool(name="p", bufs=1) as pool:
            x = pool.tile([128, SCP * SR], F32)        # [10, 260] per strip
            nc.sync.dma_start(x[:], hm.ap())

            # horizontal (column) 3-max then 5-max first — fewer elements
            # flow through the remaining cascade than vertical-first
            b1 = pool.tile([128, (SCP - 1) * SR], F32)
            b3 = pool.tile([128, (SCP - 2) * SR], F32)
            split_max((b1, 0, 0, SR, SCP - 1, SR),
                      (x, 0, 0, SR), (x, 1, 0, SR))
            split_max((b3, 0, 0, SR, SCP - 2, SR),
                      (b1, 0, 0, SR), (x, 2, 0, SR))
            h5 = pool.tile([128, SC * SR], F32)
            split_max((h5, 0, 0, SR, SC, SR),
                      (b3, 0, 0, SR), (b3, 2, 0, SR))
            # vertical (row) 3-max then 5-max on the 6 own columns
            a1 = pool.tile([128, SC * (SR - 2)], F32)
            a2 = pool.tile([128, SC * (SR - 2)], F32)
            split_max((a1, 0, 0, SR - 2, SC, SR - 2),
                      (h5, 0, 0, SR), (h5, 0, 1, SR))
            split_max((a2, 0, 0, SR - 2, SC, SR - 2),
                      (a1, 0, 0, SR - 2), (h5, 0, 2, SR))
            v5 = pool.tile([128, SC * HALF], F32)
            split_max((v5, 0, 0, HALF, SC, HALF),
                      (a2, 0, 0, SR - 2), (a2, 0, 2, SR - 2))

            # suppress own region: sup = (x_own == h5) ? x_own : 0
            # (is_equal is not a valid Pool opcode -> DVE only)
            eq = pool.tile([128, SC * HALF], F32)
            sup_a = pool.tile([128, SC * HALF], F32)
            xo = ap3(x, 2, SC, 2, HALF)
            nc.vector.tensor_tensor(ap3(eq, 0, SC, 0, HALF, HALF), xo,
                                    ap3(v5, 0, SC, 0, HALF, HALF),
                                    op=mybir.AluOpType.is_equal)
            nc.vector.tensor_mul(ap3(sup_a, 0, SC, 0, HALF, HALF),
                                 ap3(eq, 0, SC, 0, HALF, HALF), xo)
            sup_b = pool.tile([128, SC * HALF], F32)

            vt = pool.tile([128, NQ * QCAND], F32)
            it = pool.tile([128, NQ * QCAND], U16)
            for q in range(NQ):
                cur, nxt = sup_a, sup_b
                for r in range(RQ):
                    vs = vt[:, QCAND * q + 8 * r:QCAND * q + 8 * r + 8]
                    cs = cur[:, QLEN * q:QLEN * (q + 1)]
                    nc.vector.max(vs, cs)
                    nc.vector.max_index(
                        it[:, QCAND * q + 8 * r:QCAND * q + 8 * r + 8], vs, cs)
                    if r < RQ - 1:
                        nc.vector.match_replace(
                            nxt[:, QLEN * q:QLEN * (q + 1)], vs, cs, NEG)
                        cur, nxt = nxt, cur
            nc.sync.dma_start(cv.ap(), vt[:])
            nc.sync.dma_start(ci.ap(), it[:])
    nc.compile()
    return nc


def build_k2():
    nc = bacc.Bacc("TRN2", target_bir_lowering=False, debug=False, num_devices=8)
    desc = nc.dram_tensor("desc", [D, HALF * W], F32, kind="ExternalInput")
    idx = nc.dram_tensor("idx", [128, IOFF[-1]], I16, kind="ExternalInput")
    dn = nc.dram_tensor("dn", [D, NSLOT], F32, kind="ExternalOutput")

    with tile.TileContext(nc) as tc:
        with (
            tc.tile_pool(name="stream", bufs=1) as spool,
            tc.tile_pool(name="gp", bufs=2) as gpool,
            tc.tile_pool(name="fin", bufs=2) as fpool,
            tc.tile_pool(name="small", bufs=1) as pool,
            tc.tile_pool(name="psum", bufs=2, space="PSUM") as psum,
        ):
            # pull the Q7 ap_gather library in parallel with the first DMAs
            nc.gpsimd.load_library(library_config.ap_gather)
            idx_sb = pool.tile([128, IOFF[-1]], I16, tag="idx")
            nc.gpsimd.dma_start(idx_sb[:], idx.ap())

            bufs = [spool.tile([128, TPIX + 1], F32, tag=f"buf{i}",
                               name=f"buf{i}")
                    for i in range(3)]
            for bf in bufs:
                nc.vector.memset(bf[:, TPIX:TPIX + 1], 0.0)
            ones = pool.tile([D, 1], F32, tag="ones")
            nc.vector.memset(ones[:], 1.0)
            ones1 = pool.tile([1, 128], F32, tag="ones1")
            nc.vector.memset(ones1[:], 1.0)
            acc = pool.tile([D, NSLOT], F32, tag="acc")
            nc.vector.memset(acc[:], 0.0)

            def finalize(f0, f1):
                """L2-normalize final slot positions [f0, f1) and write out."""
                n = f1 - f0
                sqf = fpool.tile([D, n], F32, tag="sqf", name="sqf")
                nc.vector.tensor_mul(sqf[:], acc[:, f0:f1], acc[:, f0:f1])
                ssq_ps = psum.tile([1, n], F32, tag="ssq", name="ssqps")
                # split matmuls at PSUM bank boundaries of the psum tile
                for j in range(0, n, 512):
                    m = min(512, n - j)
                    nc.tensor.matmul(ssq_ps[:, j:j + m], ones[:],
                                     sqf[:, j:j + m], start=True, stop=True)
                ssq = fpool.tile([1, n], F32, tag="ssqs", name="ssqs")
                nc.vector.tensor_scalar_max(ssq[:], ssq_ps[:], 1e-24)
                nrm = fpool.tile([1, n], F32, tag="nrm", name="nrm")
                nc.scalar.activation(nrm[:], ssq[:],
                                     mybir.ActivationFunctionType.Sqrt)
                # one Newton step in rsqrt form (one exact recip):
                # r = recip(sqrt_lut(s)); r *= 1.5 - 0.5*s*r^2
                r0 = fpool.tile([1, n], F32, tag="r0", name="r0")
                nc.vector.reciprocal(r0[:], nrm[:])
                t1 = fpool.tile([1, n], F32, tag="t1", name="t1")
                nc.vector.tensor_mul(t1[:], r0[:], r0[:])
                nc.vector.tensor_mul(t1[:], t1[:], ssq[:])
                nc.vector.tensor_scalar(t1[:], t1[:], -0.5, 1.5,
                                        op0=mybir.AluOpType.mult,
                                        op1=mybir.AluOpType.add)
                nc.vector.tensor_mul(r0[:], r0[:], t1[:])
                rb_ps = psum.tile([D, n], F32, tag="rb", name="rbps")
                outf = fpool.tile([D, n], F32, tag="outf", name="outf")
                for j in range(0, n, 512):
                    m = min(512, n - j)
                    nc.tensor.matmul(rb_ps[:, j:j + m], ones1[:],
                                     r0[:, j:j + m], start=True, stop=True)
                    nc.vector.tensor_mul(outf[:, j:j + m],
                                         acc[:, f0 + j:f0 + j + m],
                                         rb_ps[:, j:j + m])
                nc.sync.dma_start(dn.ap()[:, f0:f1], outf[:])

            for t in range(NTILE):
                buf = bufs[t % 3]
                nc.sync.dma_start(buf[:, 0:TPIX],
                                  desc.ap()[:, t * TPIX:(t + 1) * TPIX])
                w = WSZS[t]
                g = gpool.tile([D, w], F32, tag="g", name="g")
                nc.gpsimd.ap_gather(
                    g[:], buf[:], idx_sb[:, IOFF[t]:IOFF[t + 1]],
                    channels=128, num_elems=TPIX + 1, d=1, num_idxs=w)
                o = WINS[t]
                nc.vector.tensor_add(acc[:, o:o + w], acc[:, o:o + w], g[:])
                f0, f1 = FINS[t]
                if f1 > f0:
                    finalize(f0, f1)
    nc.compile()
    return nc


_NC1 = None
_NC2 = None


def _get_ncs():
    global _NC1, _NC2
    if _NC1 is None:
        _NC1 = build_k1()
        _NC2 = build_k2()
    return _NC1, _NC2


def np_reference_single(img):
    """Full numpy reference for one image [129, 512, 768] (fallback path)."""
    desc = img[:D]
    heat = img[D]
    pad = np.full((H + 4, W + 4), -np.inf, np.float32)
    pad[2:-2, 2:-2] = heat
    pooled = np.full((H, W), -np.inf, np.float32)
    for dy in range(5):
        for dx in range(5):
            np.maximum(pooled, pad[dy:dy + H, dx:dx + W], out=pooled)
    sup = np.where(heat == pooled, heat, 0.0).astype(np.float32)
    flat = sup.reshape(-1)
    order = np.lexsort((np.arange(flat.size), -flat))[:K]
    scores = flat[order]
    y = (order // W).astype(np.int32)
    x = (order % W).astype(np.int32)
    d = desc[:, y, x].T.astype(np.float32)
    n = np.maximum(np.sqrt((d ** 2).sum(-1, keepdims=True)), 1e-12)
    return (np.stack([x, y], -1).astype(np.int32), scores.astype(np.float32),
            (d / n).astype(np.float32))


def prep_k1_inputs(unet):
    ins = []
    for k in range(8):
        b, hf = k // 2, k % 2
        r0 = HALF * hf
        hp = np.full((W + 4, SR), NEG, np.float32)
        rlo, rhi = max(0, r0 - 2), min(H, r0 + HALF + 2)
        hp[2:W + 2, rlo - (r0 - 2):rhi - (r0 - 2)] = unet[b, D, rlo:rhi, :].T
        strips = np.lib.stride_tricks.as_strided(
            hp, (128, SCP, SR), (SC * SR * 4, SR * 4, 4))
        ins.append({"hm": np.ascontiguousarray(strips).reshape(128, SCP * SR)})
    return ins


def merge_candidates(unet, k1_results):
    kps = np.zeros((4, K, 2), np.int32)
    scs = np.zeros((4, K), np.float32)
    yx = []
    for b in range(4):
        gi_all, v_all, quart_all = [], [], []
        for hf in range(2):
            res = k1_results[2 * b + hf]
            cv = res["cv"]                      # [128, 64]
            ci = res["ci"].astype(np.int64)     # idx within quarter [0, 384)
            p = np.arange(128)[:, None]
            q = np.arange(NQ * QCAND)[None, :] // QCAND
            loc = QLEN * q + ci                 # strip-local: col*256 + row
            col = SC * p + loc // HALF
            y = HALF * hf + loc % HALF
            gi_all.append((y * W + col).reshape(-1))
            v_all.append(cv.reshape(-1))
            quart_all.append(
                np.broadcast_to((p + 128 * hf) * NQ + q, cv.shape).reshape(-1))
        gi = np.concatenate(gi_all)
        v = np.concatenate(v_all)
        quart = np.concatenate(quart_all)
        pos = v > 0.0
        ok = bool(pos.sum() >= K)
        if ok:
            gi_p, v_p = gi[pos], v[pos]
            order = np.lexsort((gi_p, -v_p))[:K]
            t_k = v_p[order[-1]]
            # a quarter whose 16 extracted candidates all clear t_k might
            # have held a 17th that also would -> cannot trust
            sat = np.bincount(quart[(v >= t_k)], minlength=256 * NQ)
            ok = bool(sat.max(initial=0) < QCAND)
        if not ok:
            kp, sc, dref = np_reference_single(unet[b])
            kps[b], scs[b] = kp, sc
            yx.append((kp[:, 1].astype(np.int64), kp[:, 0].astype(np.int64),
                       False, dref))
            continue
        g = gi_p[order]
        y, x = g // W, g % W
        kps[b, :, 0], kps[b, :, 1] = x, y
        scs[b] = v_p[order]
        yx.append((y, x, True, None))
    return kps, scs, yx


def prep_k2_inputs(unet, yx):
    """Pixel-sorted, spread keypoint slots + per-tile windowed gather idx."""
    wins = np.asarray(WINS)
    wszs = np.asarray(WSZS)
    ins, slot_maps = [], []
    for k in range(8):
        b, hf = k // 2, k % 2
        y, x, ok, _ = yx[b]
        r0 = HALF * hf
        dsc = np.ascontiguousarray(
            unet[b, :D, r0:r0 + HALF, :].reshape(D, HALF * W))
        lidx_arr = np.full(NSLOT, -1, np.int64)
        rank_arr = np.full(NSLOT, -1, np.int64)
        if ok:
            ranks = np.where((y >= r0) & (y < r0 + HALF))[0]
            lidx = (y[ranks] - r0) * W + x[ranks]
            order = np.argsort(lidx, kind="stable")
            lidx, ranks = lidx[order], ranks[order]
            n = len(ranks)
            cov = n <= NSLOT
            if cov:
                pos = (np.arange(n) * NSLOT) // max(n, 1)
                lidx_arr[pos] = lidx
                rank_arr[pos] = ranks
                tjv = lidx // TPIX
                cov = bool(np.all((wins[tjv] <= pos) &
                                  (pos < wins[tjv] + wszs[tjv])))
            if not cov:        # window overflow (~1e-5/run) -> exact fallback
                kp, sc, dref = np_reference_single(unet[b])
                yx[b] = (kp[:, 1].astype(np.int64), kp[:, 0].astype(np.int64),
                         False, dref)
                lidx_arr[:] = -1
                rank_arr[:] = -1
        wrapped = []
        for t in range(NTILE):
            wj = lidx_arr[WINS[t]:WINS[t] + WSZS[t]]
            lt = wj - TPIX * t
            arr = np.where((wj >= 0) & (lt >= 0) & (lt < TPIX), lt, TPIX)
            wrapped.append(arr.reshape(IWS[t], 16).T)       # [16, IW_t]
        w16 = np.concatenate(wrapped, axis=1)               # [16, IOFF[-1]]
        idx16 = np.zeros((128, IOFF[-1]), np.int16)
        for grp in range(8):
            idx16[16 * grp:16 * grp + 16] = w16
        ins.append({"desc": dsc, "idx": idx16})
        slot_maps.append(rank_arr)
    return ins, slot_maps


def kernel(unet_output):
    unet = np.asarray(unet_output, dtype=np.float32)
    assert unet.shape == (4, D + 1, H, W), unet.shape
    nc1, nc2 = _get_ncs()
    cores = list(range(8))

    k1_ins = prep_k1_inputs(unet)
    r1 = run_bass_kernel_spmd(nc1, k1_ins, cores, trace=_PROFILE)
    if _PROFILE:
        _EXEC_NS["k1"] = r1.exec_time_ns
    kps, scs, yx = merge_candidates(unet, r1.results)

    k2_ins, slot_maps = prep_k2_inputs(unet, yx)
    r2 = run_bass_kernel_spmd(nc2, k2_ins, cores, trace=_PROFILE)
    if _PROFILE:
        _EXEC_NS["k2"] = r2.exec_time_ns

    d_out = np.zeros((4, K, D), np.float32)
    for b in range(4):
        y, x, ok, dref = yx[b]
        if not ok:
            d_out[b] = dref
            continue
        for hf in range(2):
            k = 2 * b + hf
            rank_arr = slot_maps[k]
            v = rank_arr >= 0
            dnb = r2.results[k]["dn"]
            d_out[b, rank_arr[v], :] = dnb[:, v].T
    return kps, scs, d_out
